# revision 58
# baseline (speedup 1.0000x reference)
"""Cross-attention Trainium2 Bass kernel.

Problem (per full input):
    q_in [8, 2048, 1024] f32, k_v [8, 2048, 1024] f32,
    Wq/Wk/Wv [1024, 1024] f32, bq/bk/bv [1024] f32
    q = q_in @ Wq + bq; k = k_v @ Wk + bk; v = k_v @ Wv + bv
    out = softmax(q k^T / sqrt(1024)) v        -> [8, 2048, 1024] f32

Sharding: data-parallel over batch, one batch per NeuronCore (8 cores).

Key algebraic reduction: q and k only ever appear through
    sim = (x_q Wq + bq)(x_k Wk + bk)^T
       = x_q (Wq Wk^T) x_k^T  +  [per-i shift, cancels in softmax]
         + (x_k Wk bq)_j      +  [const, cancels]
so with M := Wq Wk^T precomputed on the host (weight-only, O(E^3)) the
kernel needs just ONE projection k' = M x_k^T instead of separate q and
k projections — 2.15e9 of the 15e9 per-core MACs disappear.  The per-j
bias term beta_j = (x_k Wk bq)_j (zero for this problem's zero biases,
but handled generally) folds into the exp activation's per-partition
bias.  x_q feeds the attention matmul directly from HBM.

Per-core algorithm (I = J = 2048, E = D = 1024, P = 128):
  - Host pre-transposes activations to [E, I] and casts to fp16.
  - k'T[d,j] computed with the M chunk as the stationary operand (output
    comes out transposed, exactly the layout the attention matmul
    needs); v[j,e] computed with the x_kT chunk stationary.
  - Attention: simT[j,i] = k'T^T x_qT accumulated over d in PSUM; exp on
    the ACT engine with the 1/sqrt(E) scale and beta bias fused; PV
    accumulates sum_j expT[j,i] v[j,e] over all j in PSUM (unnormalized).
    Each v chunk is padded with two ones columns so the softmax
    denominator rides along inside the PV accumulation — PV is split
    384/384/258 (a 514-wide f32 tile would overflow the 2KB PSUM bank)
    and the last piece's final columns accumulate sum_j expT.
    (Separate N=1 den matmuls cost ~32ns of PE issue turnaround each,
    8us total.)  A per-partition reciprocal multiply normalizes at
    eviction.
  - exp is computed without max subtraction: sim ~ N(0,1) for this
    problem's distribution, so exp() stays comfortably inside fp16/fp32
    range and softmax is shift-invariant anyway.
  - Output is evicted and DMA'd as fp16 (rounding ~2.4e-4 relative, far
    under the 2e-2 gate); the host upcasts to fp32.
  - fp8 was evaluated and rejected: e4m3's 3 mantissa bits measure
    3e-2..6e-2 on the max-norm metric for any of sim/PV quantized
    (numpy study on the real data), over the 2e-2 gate.
"""

import numpy as np
from contextlib import ExitStack

import concourse.bass as bass
import concourse.mybir as mybir
import concourse.tile as tile
from concourse import bacc
from concourse.bass_utils import run_bass_kernel_spmd

B = 8
I = 2048  # query positions per batch
J = 2048  # key positions per batch
E = 1024  # embed dim
P = 128
EC = E // P  # 8 contraction chunks
SCALE = float(E) ** -0.5

F16 = mybir.dt.float16
F32 = mybir.dt.float32

# i-block size for the attention phase (sim moving free dim).  256 keeps the
# PSUM budget at 8 banks: 2 i-subtiles x 3 PV pieces + 2 simT.
IB = 256

# Module-level knobs test.py may override before the first kernel() call.
_RUN_KWARGS: dict = {}
LAST_RESULTS = None

_NC_CACHE: dict = {}


def _build():
    nc = bacc.Bacc("TRN2", target_bir_lowering=False, debug=False)

    q_inT = nc.dram_tensor("q_inT", [E, I], F16, kind="ExternalInput")
    k_vT = nc.dram_tensor("k_vT", [E, J], F16, kind="ExternalInput")
    M_t = nc.dram_tensor("M_t", [E, E], F16, kind="ExternalInput")
    Wv_d = nc.dram_tensor("Wv", [E, E], F16, kind="ExternalInput")
    # beta[p, jc]: SCALE * (x_k Wk bq)_j at j = jc*128 + p, fused into exp
    beta_d = nc.dram_tensor("beta_t", [P, J // P], F32, kind="ExternalInput")
    bv_bc = nc.dram_tensor("bv_bc", [P, E], F32, kind="ExternalInput")
    out_d = nc.dram_tensor("out", [I, E], F16, kind="ExternalOutput")

    with tile.TileContext(nc) as tc, ExitStack() as ctx:
        const = ctx.enter_context(tc.tile_pool(name="const", bufs=1))
        beta_sb = const.tile([P, J // P], F32, tag="beta")
        bv_sb = const.tile([P, E], F32, tag="bv")

        # Persistent fp16 operands for the attention phase.
        # xqT/kT: chunk d lives at [:, d*I + i]  (layout [d, i] / [d, j])
        # v:     chunk jc lives at [:, jc*E + e] (layout [j, e])
        persist = ctx.enter_context(tc.tile_pool(name="persist", bufs=1))
        qT_sb = persist.tile([P, EC * I], F16, tag="qT")
        kT_sb = persist.tile([P, EC * J], F16, tag="kT")
        # v chunks padded to E+2 columns: cols E and E+1 hold 1.0 so the
        # softmax denominator rides along inside the PV accumulation (the
        # N=1 den matmuls cost ~32ns of PE issue time each; folding them
        # into a PV piece makes them free).  Both ones columns accumulate
        # the identical denominator; the second pads the den piece to an
        # even 258.  (The ~+3ns/instr measured on that piece turned out to
        # be accumulation-group-end overhead, not odd-width — 257 and 258
        # measure the same — so the pad is merely harmless.)
        NONES = 2
        EV = E + NONES
        v_sb = persist.tile([P, (J // P) * EV], F16, tag="v")
        for jc in range(J // P):
            nc.vector.memset(v_sb[:, jc * EV + E : (jc + 1) * EV], 1.0)

        # sim/exp live at context scope so the first attention groups can be
        # pre-emitted into the projection tail (PSUM: sim 2 + proj 5 + warm 1
        # = 8 banks during projections; sim 2 + pv 6 = 8 during attention).
        sim_ps_pool = ctx.enter_context(
            tc.tile_pool(name="sim_ps", bufs=2, space="PSUM")
        )
        exp_pool = ctx.enter_context(tc.tile_pool(name="exp", bufs=4))

        def emit_sim_exp(i0, ibsz, jc):
            sim = sim_ps_pool.tile([P, IB], F32, tag="sim",
                                   name=f"sim_{i0}_{jc}")
            for d in range(EC):
                nc.tensor.matmul(
                    sim[:, 0:ibsz],
                    kT_sb[:, d * J + jc * P : d * J + (jc + 1) * P],
                    qT_sb[:, d * I + i0 : d * I + i0 + ibsz],
                    start=(d == 0),
                    stop=(d == EC - 1),
                )
            expT = exp_pool.tile([P, IB], F16, tag="expT")
            nc.scalar.activation(
                expT[:, 0:ibsz], sim[:, 0:ibsz],
                mybir.ActivationFunctionType.Exp,
                scale=SCALE,
                bias=beta_sb[:, jc : jc + 1],
            )
            return expT

        # ---------------- phase A/B: projections ----------------
        with ExitStack() as ab:
            wpool = ab.enter_context(tc.tile_pool(name="wpool", bufs=1))
            # Both weight matrices in one tile: W chunk e at
            # [:, w_off + e*E + c]   ([128, 16384] f16 = 32KB/partition).
            # Per-chunk 512-col DMAs interleaved with the x chunks measured
            # best: batching waves into single large DMAs (tried, reverted)
            # turns the wave into an all-or-nothing barrier and idles the PE
            # ~7-11us during the clock ramp, when DMA delivers slowly and
            # the per-e semaphore gating of the accumulation loop is what
            # keeps the PE fed.
            w_sb = wpool.tile([P, 2 * EC * E], F16, tag="W")
            w_off = {"M": 0, "Wv": EC * E}
            w_dram = {"M": M_t, "Wv": Wv_d}

            def wcol(wname, e, c):
                # column of W chunk-e col-c in w_sb
                return w_off[wname] + e * E + c

            def load_w_chunk(w, e, eng=None):
                # M rides the scalar engine's HWDGE queue, Wv the sync
                # queue, so both stream in parallel with the activation
                # chunks.  All weight issues happen in the first ~10us,
                # while those engines have no other work yet — DMA_DIRECT2D
                # costs ~600ns of issue time on the queueing engine, so it
                # must never sit in front of exp/eviction work.
                (eng or nc.scalar).dma_start(
                    w_sb[:, w_off[w] + e * E : w_off[w] + (e + 1) * E],
                    w_dram[w].ap()[e * P : (e + 1) * P, :],
                )

            xpool = ab.enter_context(tc.tile_pool(name="xpool", bufs=2))
            ppool = ab.enter_context(
                tc.tile_pool(name="proj_ps", bufs=5, space="PSUM")
            )

            H = 1024  # half of the j range handled per streamed xT tile

            def xcol(e, c):
                # column of x chunk-e col-c in xh
                return e * H + c

            def load_half(src, h, with_w=None):
                xh = xpool.tile([P, EC * H], F16, tag="xT")
                if with_w is not None:
                    # Startup ramp: deliver in first-touch order at 512-col
                    # granularity so the first 2MB (x cols 0:512 on sync + M
                    # cols 0:512 on scalar) unlocks the first 4 PSUM groups
                    # (~7us of PE work) while the second waves stream behind.
                    # x-h0 delivery is DMA-issue-rate-bound (~600ns/issue),
                    # so the last-consumed chunks e6/e7 ride the gpsimd SWDGE
                    # queue, whose slow (~12us) bring-up matches when the
                    # in-order e-loop reaches them.
                    # 512-col wave granularity on M measured best: 256-col
                    # sub-waves (tried, reverted) double the scalar queue's
                    # ramp-window issue count, and at 64KB per ~0.6us issue
                    # the queue becomes issue-rate-capped (~107 GB/s) — a
                    # net +5us loss despite the earlier (d0-1,ib0) start.
                    for e in range(EC):
                        nc.scalar.dma_start(
                            w_sb[:, w_off[with_w] + e * E
                                 : w_off[with_w] + e * E + 512],
                            w_dram[with_w].ap()[e * P : (e + 1) * P, 0:512],
                        )
                        # e6/e7 ride gpsimd at the same 512-col wave
                        # granularity as the sync chunks: a full-H chunk
                        # makes the wave-1 groups wait for cols 512:1024
                        # they don't need yet (~1.3us PE gap each).
                        (nc.sync if e < EC - 2 else nc.gpsimd).dma_start(
                            xh[:, e * H : e * H + 512],
                            src.ap()[e * P : (e + 1) * P,
                                     h * H : h * H + 512],
                        )
                    for e in range(EC):
                        nc.scalar.dma_start(
                            w_sb[:, w_off[with_w] + e * E + 512
                                 : w_off[with_w] + (e + 1) * E],
                            w_dram[with_w].ap()[e * P : (e + 1) * P, 512:E],
                        )
                        (nc.sync if e < EC - 2 else nc.gpsimd).dma_start(
                            xh[:, e * H + 512 : (e + 1) * H],
                            src.ap()[e * P : (e + 1) * P,
                                     h * H + 512 : (h + 1) * H],
                        )
                else:
                    for e in range(EC):
                        nc.sync.dma_start(
                            xh[:, e * H : (e + 1) * H],
                            src.ap()[e * P : (e + 1) * P, h * H : (h + 1) * H],
                        )
                return xh

            def load_qT(ec_range, eng):
                # x_q needs no projection: DMA it straight into the
                # attention-phase operand slot.  Queued AFTER the
                # projection-critical bytes (HBM bandwidth is shared across
                # queues; order is the only priority control).
                for e in range(*ec_range):
                    eng.dma_start(
                        qT_sb[:, e * I : (e + 1) * I],
                        q_inT.ap()[e * P : (e + 1) * P, :],
                    )

            def proj_T(xh, h, wname, dst, order=None):
                # dst[d, n] = sum_e W[e,d] x[n,e], n in this half
                if order is None:
                    order = [(d, ib) for d in range(EC)
                             for ib in range(H // 512)]
                for d, ib in order:
                    if True:
                        ps = ppool.tile([P, 512], F32, tag="proj")
                        for e in range(EC):
                            nc.tensor.matmul(
                                ps[:],
                                w_sb[:, wcol(wname, e, d * P)
                                     : wcol(wname, e, d * P) + P],
                                xh[:, xcol(e, ib * 512)
                                   : xcol(e, ib * 512) + 512],
                                start=(e == 0),
                                stop=(e == EC - 1),
                            )
                        nc.scalar.activation(
                            dst[:, d * I + h * H + ib * 512
                                : d * I + h * H + (ib + 1) * 512],
                            ps[:],
                            mybir.ActivationFunctionType.Identity,
                        )

            def proj_v(xh, h):
                # v[j, e] = sum_e' k_v[j, e'] Wv[e', e] + bv[e], j in this half
                for jc in range(H // P):
                    jg = h * (H // P) + jc
                    for eh in range(E // 512):
                        ps = ppool.tile([P, 512], F32, tag="proj")
                        for e in range(EC):
                            nc.tensor.matmul(
                                ps[:],
                                xh[:, xcol(e, jc * P)
                                   : xcol(e, jc * P) + P],
                                w_sb[:, wcol("Wv", e, eh * 512)
                                     : wcol("Wv", e, eh * 512) + 512],
                                start=(e == 0),
                                stop=(e == EC - 1),
                            )
                        nc.vector.tensor_add(
                            v_sb[:, jg * EV + eh * 512 : jg * EV + eh * 512 + 512],
                            ps[:],
                            bv_sb[:, eh * 512 : (eh + 1) * 512],
                        )

            # Warmup spin: throwaway matmuls on a zeroed tile keep the PE
            # busy from ~7.5us (engine start) so the HAM activity window
            # fills and the clock gate opens right as the first real
            # operands land (the runtime preamble plus first-chunk transfer
            # make ~13us the floor).  Without this the first ~25us of
            # projections run at half clock.  NOTE: the device's steady
            # clock itself varies per run (measured 1.98 vs 2.37 GHz with
            # identical code — ~70us swing); that state is not controllable
            # from the kernel.
            warm_ps_pool = ab.enter_context(
                tc.tile_pool(name="warm_ps", bufs=1, space="PSUM")
            )
            warm_sb = const.tile([P, 512], F16, tag="warm")
            nc.vector.memset(warm_sb[:], 0.0)
            warm_ps = warm_ps_pool.tile([P, 512], F32, tag="warm")

            def spin(n):
                for _ in range(n):
                    nc.tensor.matmul(
                        warm_ps[:], warm_sb[:, 0:P], warm_sb[:],
                        start=True, stop=True, skip_group_check=True,
                    )

            # DMA priority (HBM bandwidth is shared across queues, FIFO within
            # one): the critical first set — x-h0 on sync+gpsimd + M on scalar
            # — owns the queues from t=0.  Everything else follows in deadline
            # order.  bv (512KB, needed ~65us) queues BEHIND the ramp-critical
            # e6/7 chunks on gpsimd.  proj_T(h1) runs BEFORE proj_v(h0) so the
            # Wv deadline moves from ~35us to ~62us, letting Wv queue behind
            # x-h1.
            xh0 = load_half(k_vT, 0, with_w="M")
            # beta (8KB, needed at first exp ~100us) and bv (512KB, needed
            # ~65us) queue BEHIND the ramp-critical e6/7 chunks on gpsimd —
            # even beta's ~780ns SWDGE issue in front of e6 costs PE time.
            nc.gpsimd.dma_start(beta_sb[:], beta_d.ap())
            nc.gpsimd.dma_start(bv_sb[:], bv_bc.ap())
            xh1 = load_half(k_vT, 1)
            for e in range(EC):
                load_w_chunk("Wv", e, eng=nc.sync)
            load_qT((0, EC), nc.sync)
            # h0 is emitted e-INNER across four simultaneously-open PSUM
            # groups, in data-arrival order (wave 1 = x cols 0:512 + M cols
            # 0:512 serves (d0-3, ib0); wave 2 adds x 512:1024 -> ib1 and M
            # 512:1024 -> d4-7).  Each arriving x/M e-chunk pair immediately
            # yields 4 real matmuls (2048 cycles), so the wave-1 work
            # retires DURING the DMA-bound ramp window instead of after it.
            # Interleaved spins keep the HAM activity duty high between
            # chunk arrivals so the DVFS gate, opened by the lead-in spins
            # at ~13-14us, never re-closes (a closed gate replays matmuls
            # at half clock even with data present).
            def proj_T_set(xh, h, wname, dst, ds, ib, spins_per_e):
                ps = [
                    ppool.tile([P, 512], F32, tag="proj",
                               name=f"ps_{h}_{ib}_{d}")
                    for d in ds
                ]
                for e in range(EC):
                    for i, d in enumerate(ds):
                        nc.tensor.matmul(
                            ps[i][:],
                            w_sb[:, wcol(wname, e, d * P)
                                 : wcol(wname, e, d * P) + P],
                            xh[:, xcol(e, ib * 512)
                               : xcol(e, ib * 512) + 512],
                            start=(e == 0),
                            stop=(e == EC - 1),
                        )
                    if spins_per_e:
                        spin(spins_per_e)
                for i, d in enumerate(ds):
                    nc.scalar.activation(
                        dst[:, d * I + h * H + ib * 512
                            : d * I + h * H + (ib + 1) * 512],
                        ps[i][:],
                        mybir.ActivationFunctionType.Identity,
                    )

            spin(13)
            proj_T_set(xh0, 0, "M", kT_sb, (0, 1, 2, 3), 0, 0)
            proj_T_set(xh0, 0, "M", kT_sb, (0, 1, 2, 3), 1, 0)
            proj_T_set(xh0, 0, "M", kT_sb, (4, 5, 6, 7), 0, 0)
            proj_T_set(xh0, 0, "M", kT_sb, (4, 5, 6, 7), 1, 0)
            proj_T(xh1, 1, "M", kT_sb)
            proj_v(xh0, 0)
            # Pre-emit the first attention block's first two sim+exp groups
            # into the projection tail (they need only kT h0 + qT, both
            # resident) — primes the exp pipeline so attention opens with
            # its PSUM banks already drained and no transition bubble.
            primed = [(jc, emit_sim_exp(0, IB, jc)) for jc in range(2)]
            proj_v(xh1, 1)

        # ---------------- phase C: attention ----------------
        with ExitStack() as c:
            # NOTE: matmul start=True clears has_written for the WHOLE PSUM
            # bank, so each accumulation group needs its own bank.  PV is
            # split 384/384/257 (not 512/512) so the denominator's ones
            # column fits the 2KB bank (513*4 would not); 2 isub * 3 pieces
            # = 6 banks + 2 sim = all 8.
            pv_ps_pool = c.enter_context(
                tc.tile_pool(name="pv_ps", bufs=6, space="PSUM")
            )
            out_pool = c.enter_context(tc.tile_pool(name="outsb", bufs=6))
            small = c.enter_context(tc.tile_pool(name="small", bufs=2))

            # (col0, width) of each PV piece within the padded EV-col v
            # chunk; the last piece's final column is the ones column, so
            # its last PSUM col accumulates the softmax denominator.
            # (A 4-piece final block was tried to narrow the last drain:
            # the extra evict+DMA pair contends on the two drain queues and
            # issues the last DMA ~1us LATER — 3 pieces measured best.)
            PW3 = [(0, 384), (384, 384), (768, 258)]
            NJC = J // P
            # The final 256-row block splits into two 128-row blocks: the
            # penultimate one's drain overlaps the last one's compute, and
            # the unavoidable serial tail chain halves.
            blocks = [(k * IB, IB) for k in range(I // IB - 1)]
            blocks += [(I - IB, P), (I - P, P)]

            for bi, (i0, ibsz) in enumerate(blocks):
                nsub = ibsz // P
                last = bi == len(blocks) - 1
                PW = PW3
                dp = len(PW) - 1  # den-carrying piece index
                dcol = PW[dp][1] - 1  # den column within that piece
                pv = [
                    [
                        pv_ps_pool.tile(
                            [P, w], F32, tag="pv", name=f"pv_{i0}_{s}_{pi}"
                        )
                        for pi, (_, w) in enumerate(PW)
                    ]
                    for s in range(nsub)
                ]

                def emit_pv(jc, expT):
                    for isub in range(nsub):
                        lhs = expT[:, isub * P : (isub + 1) * P]
                        # On the final j-chunk, finish the den-carrying
                        # piece FIRST so the recip -> eviction -> DMA drain
                        # chain overlaps the other PV streams instead of
                        # trailing them.
                        order = range(len(PW))
                        if jc == NJC - 1:
                            order = (dp, *range(dp))
                        for pi in order:
                            c0, w = PW[pi]
                            nc.tensor.matmul(
                                pv[isub][pi][:],
                                lhs,
                                v_sb[:, jc * EV + c0 : jc * EV + c0 + w],
                                start=(jc == 0),
                                stop=(jc == NJC - 1),
                            )

                # pv/den for chunk jc are emitted after sim for chunk
                # jc+2, so the exp -> semaphore -> PE latency hides under
                # two full sim streams instead of poking a ~125ns bubble
                # into each cycle.  Block 0's first two groups were
                # pre-emitted into the projection tail.
                pending = list(primed) if bi == 0 else []
                for jc in range(len(pending), NJC):
                    expT = emit_sim_exp(i0, ibsz, jc)
                    pending.append((jc, expT))
                    if len(pending) > 2:
                        emit_pv(*pending.pop(0))
                for item in pending:
                    emit_pv(*item)

                recip = small.tile([P, nsub], F32, tag="recip")
                for isub in range(nsub):
                    nc.vector.reciprocal(
                        recip[:, isub : isub + 1],
                        pv[isub][dp][:, dcol : dcol + 1],
                    )
                # All evictions on DVE (idle during attention) so the ACT
                # queue stays exp-only — an eviction burst on ACT at a block
                # boundary delays exp(0) and stalls the PE on the sim-bank
                # reuse semaphore.  Exception: the final block has no exp
                # work left, so its evictions alternate DVE/ACT and each
                # piece kicks its own DMA (sync/scalar) as soon as it lands,
                # in piece-completion order (dp first, narrow piece last).
                for isub in range(nsub):
                    if not last:
                        o = out_pool.tile([P, E], F16, tag="o")
                        for pi, (c0, w) in enumerate(PW):
                            wo = w - NONES if pi == dp else w
                            nc.vector.tensor_scalar_mul(
                                o[:, c0 : c0 + wo],
                                pv[isub][pi][:, 0:wo],
                                recip[:, isub : isub + 1],
                            )
                        nc.sync.dma_start(
                            out_d.ap()[i0 + isub * P : i0 + (isub + 1) * P, :],
                            o[:],
                        )
                    else:
                        # Three fully parallel drain lanes — DVE+sync,
                        # ACT+scalar, gpsimd+gpsimd (all idle here) — so
                        # the last-finishing piece's evict+DMA never queues
                        # behind an earlier piece's on a shared engine.
                        for k, pi in enumerate((dp, *range(dp))):
                            c0, w = PW[pi]
                            wo = w - NONES if pi == dp else w
                            o = out_pool.tile([P, 384], F16, tag="o2")
                            # gpsimd cannot read PSUM, so evictions stay on
                            # DVE/ACT (the two DVE ones stagger enough to
                            # run back-to-back); only the DMA rides gpsimd.
                            if k == 1:
                                nc.scalar.activation(
                                    o[:, 0:wo],
                                    pv[isub][pi][:, 0:wo],
                                    mybir.ActivationFunctionType.Copy,
                                    scale=recip[:, isub : isub + 1],
                                )
                            else:
                                nc.vector.tensor_scalar_mul(
                                    o[:, 0:wo], pv[isub][pi][:, 0:wo],
                                    recip[:, isub : isub + 1],
                                )
                            (nc.sync, nc.scalar, nc.gpsimd)[k].dma_start(
                                out_d.ap()[
                                    i0 + isub * P : i0 + (isub + 1) * P,
                                    c0 : c0 + wo,
                                ],
                                o[:, 0:wo],
                            )

    nc.compile()
    return nc


def _get_nc():
    if "nc" not in _NC_CACHE:
        _NC_CACHE["nc"] = _build()
    return _NC_CACHE["nc"]


def kernel(q_in, k_v, Wq, bq, Wk, bk, Wv, bv):
    q_in = np.asarray(q_in, dtype=np.float32)
    k_v = np.asarray(k_v, dtype=np.float32)
    Wq = np.asarray(Wq, dtype=np.float32)
    Wk = np.asarray(Wk, dtype=np.float32)
    Wv = np.asarray(Wv, dtype=np.float32)
    bq = np.asarray(bq, dtype=np.float32)
    bv = np.asarray(bv, dtype=np.float32)

    nc = _get_nc()

    # sim = x_q (Wq Wk^T) x_k^T (+ bias terms, see module docstring).
    # proj_T consumes weights in [in, out] layout: W'[e, d] with
    # k'[d] = sum_e W'[e, d] x_k[e], and W' = (Wq Wk^T)^T = Wk Wq^T.
    M_t16 = np.ascontiguousarray((Wk @ Wq.T).astype(np.float16))
    Wv16 = np.ascontiguousarray(Wv.astype(np.float16))
    bv_bc = np.ascontiguousarray(np.broadcast_to(bv, (P, E)))
    u = Wk @ bq  # beta_j = SCALE * x_k[j] . u  (zero when bq == 0)

    in_maps = []
    for b in range(B):
        beta = (SCALE * (k_v[b] @ u)).astype(np.float32)
        beta_t = np.ascontiguousarray(beta.reshape(J // P, P).T)
        in_maps.append(
            {
                "q_inT": np.ascontiguousarray(q_in[b].T).astype(np.float16),
                "k_vT": np.ascontiguousarray(k_v[b].T).astype(np.float16),
                "M_t": M_t16,
                "Wv": Wv16,
                "beta_t": beta_t,
                "bv_bc": bv_bc,
            }
        )

    global LAST_RESULTS
    LAST_RESULTS = run_bass_kernel_spmd(
        nc, in_maps, core_ids=list(range(B)), **_RUN_KWARGS
    )
    return np.stack(
        [LAST_RESULTS.results[b]["out"].astype(np.float32) for b in range(B)]
    )



# revision 59
# speedup vs baseline: 1.0072x; 1.0072x over previous
"""Cross-attention Trainium2 Bass kernel.

Problem (per full input):
    q_in [8, 2048, 1024] f32, k_v [8, 2048, 1024] f32,
    Wq/Wk/Wv [1024, 1024] f32, bq/bk/bv [1024] f32
    q = q_in @ Wq + bq; k = k_v @ Wk + bk; v = k_v @ Wv + bv
    out = softmax(q k^T / sqrt(1024)) v        -> [8, 2048, 1024] f32

Sharding: data-parallel over batch, one batch per NeuronCore (8 cores).

Key algebraic reduction: q and k only ever appear through
    sim = (x_q Wq + bq)(x_k Wk + bk)^T
       = x_q (Wq Wk^T) x_k^T  +  [per-i shift, cancels in softmax]
         + (x_k Wk bq)_j      +  [const, cancels]
so with M := Wq Wk^T precomputed on the host (weight-only, O(E^3)) the
kernel needs just ONE projection k' = M x_k^T instead of separate q and
k projections — 2.15e9 of the 15e9 per-core MACs disappear.  The per-j
bias term beta_j = (x_k Wk bq)_j (zero for this problem's zero biases,
but handled generally) folds into the exp activation's per-partition
bias.  x_q feeds the attention matmul directly from HBM.

Per-core algorithm (I = J = 2048, E = D = 1024, P = 128):
  - Host pre-transposes activations to [E, I] and casts to fp16.
  - k'T[d,j] computed with the M chunk as the stationary operand (output
    comes out transposed, exactly the layout the attention matmul
    needs); v[j,e] computed with the x_kT chunk stationary.
  - Attention: simT[j,i] = k'T^T x_qT accumulated over d in PSUM; exp on
    the ACT engine with the 1/sqrt(E) scale and beta bias fused; PV
    accumulates sum_j expT[j,i] v[j,e] over all j in PSUM (unnormalized).
    Each v chunk is padded with two ones columns so the softmax
    denominator rides along inside the PV accumulation — PV is split
    384/384/258 (a 514-wide f32 tile would overflow the 2KB PSUM bank)
    and the last piece's final columns accumulate sum_j expT.
    (Separate N=1 den matmuls cost ~32ns of PE issue turnaround each,
    8us total.)  A per-partition reciprocal multiply normalizes at
    eviction.
  - exp is computed without max subtraction: sim ~ N(0,1) for this
    problem's distribution, so exp() stays comfortably inside fp16/fp32
    range and softmax is shift-invariant anyway.
  - Output is evicted and DMA'd as fp16 (rounding ~2.4e-4 relative, far
    under the 2e-2 gate); the host upcasts to fp32.
  - fp8 was evaluated and rejected: e4m3's 3 mantissa bits measure
    3e-2..6e-2 on the max-norm metric for any of sim/PV quantized
    (numpy study on the real data), over the 2e-2 gate.
"""

import numpy as np
from contextlib import ExitStack

import concourse.bass as bass
import concourse.mybir as mybir
import concourse.tile as tile
from concourse import bacc
from concourse.bass_utils import run_bass_kernel_spmd

B = 8
I = 2048  # query positions per batch
J = 2048  # key positions per batch
E = 1024  # embed dim
P = 128
EC = E // P  # 8 contraction chunks
SCALE = float(E) ** -0.5

F16 = mybir.dt.float16
F32 = mybir.dt.float32

# i-block size for the attention phase (sim moving free dim).  256 keeps the
# PSUM budget at 8 banks: 2 i-subtiles x 3 PV pieces + 2 simT.
IB = 256

# Module-level knobs test.py may override before the first kernel() call.
_RUN_KWARGS: dict = {}
LAST_RESULTS = None

_NC_CACHE: dict = {}


def _build():
    nc = bacc.Bacc("TRN2", target_bir_lowering=False, debug=False)

    q_inT = nc.dram_tensor("q_inT", [E, I], F16, kind="ExternalInput")
    k_vT = nc.dram_tensor("k_vT", [E, J], F16, kind="ExternalInput")
    M_t = nc.dram_tensor("M_t", [E, E], F16, kind="ExternalInput")
    Wv_d = nc.dram_tensor("Wv", [E, E], F16, kind="ExternalInput")
    # beta[p, jc]: SCALE * (x_k Wk bq)_j at j = jc*128 + p, fused into exp
    beta_d = nc.dram_tensor("beta_t", [P, J // P], F32, kind="ExternalInput")
    bv_bc = nc.dram_tensor("bv_bc", [P, E], F32, kind="ExternalInput")
    out_d = nc.dram_tensor("out", [I, E], F16, kind="ExternalOutput")

    with tile.TileContext(nc) as tc, ExitStack() as ctx:
        const = ctx.enter_context(tc.tile_pool(name="const", bufs=1))
        beta_sb = const.tile([P, J // P], F32, tag="beta")
        bv_sb = const.tile([P, E], F32, tag="bv")

        # Persistent fp16 operands for the attention phase.
        # xqT/kT: chunk d lives at [:, d*I + i]  (layout [d, i] / [d, j])
        # v:     chunk jc lives at [:, jc*E + e] (layout [j, e])
        persist = ctx.enter_context(tc.tile_pool(name="persist", bufs=1))
        qT_sb = persist.tile([P, EC * I], F16, tag="qT")
        kT_sb = persist.tile([P, EC * J], F16, tag="kT")
        # v chunks padded to E+2 columns: cols E and E+1 hold 1.0 so the
        # softmax denominator rides along inside the PV accumulation (the
        # N=1 den matmuls cost ~32ns of PE issue time each; folding them
        # into a PV piece makes them free).  Both ones columns accumulate
        # the identical denominator; the second pads the den piece to an
        # even 258.  (The ~+3ns/instr measured on that piece turned out to
        # be accumulation-group-end overhead, not odd-width — 257 and 258
        # measure the same — so the pad is merely harmless.)
        NONES = 2
        EV = E + NONES
        v_sb = persist.tile([P, (J // P) * EV], F16, tag="v")
        for jc in range(J // P):
            nc.vector.memset(v_sb[:, jc * EV + E : (jc + 1) * EV], 1.0)

        # sim/exp live at context scope so the first attention groups can be
        # pre-emitted into the projection tail (PSUM: sim 2 + proj 5 + warm 1
        # = 8 banks during projections; sim 2 + pv 6 = 8 during attention).
        sim_ps_pool = ctx.enter_context(
            tc.tile_pool(name="sim_ps", bufs=2, space="PSUM")
        )
        exp_pool = ctx.enter_context(tc.tile_pool(name="exp", bufs=4))

        def emit_sim_exp(i0, ibsz, jc):
            sim = sim_ps_pool.tile([P, IB], F32, tag="sim",
                                   name=f"sim_{i0}_{jc}")
            for d in range(EC):
                nc.tensor.matmul(
                    sim[:, 0:ibsz],
                    kT_sb[:, d * J + jc * P : d * J + (jc + 1) * P],
                    qT_sb[:, d * I + i0 : d * I + i0 + ibsz],
                    start=(d == 0),
                    stop=(d == EC - 1),
                )
            expT = exp_pool.tile([P, IB], F16, tag="expT")
            nc.scalar.activation(
                expT[:, 0:ibsz], sim[:, 0:ibsz],
                mybir.ActivationFunctionType.Exp,
                scale=SCALE,
                bias=beta_sb[:, jc : jc + 1],
            )
            return expT

        # ---------------- phase A/B: projections ----------------
        with ExitStack() as ab:
            wpool = ab.enter_context(tc.tile_pool(name="wpool", bufs=1))
            # Both weight matrices in one tile: W chunk e at
            # [:, w_off + e*E + c]   ([128, 16384] f16 = 32KB/partition).
            # Per-chunk 512-col DMAs interleaved with the x chunks measured
            # best: batching waves into single large DMAs (tried, reverted)
            # turns the wave into an all-or-nothing barrier and idles the PE
            # ~7-11us during the clock ramp, when DMA delivers slowly and
            # the per-e semaphore gating of the accumulation loop is what
            # keeps the PE fed.
            w_sb = wpool.tile([P, 2 * EC * E], F16, tag="W")
            w_off = {"M": 0, "Wv": EC * E}
            w_dram = {"M": M_t, "Wv": Wv_d}

            def wcol(wname, e, c):
                # column of W chunk-e col-c in w_sb
                return w_off[wname] + e * E + c

            def load_w_chunk(w, e, eng=None):
                # M rides the scalar engine's HWDGE queue, Wv the sync
                # queue, so both stream in parallel with the activation
                # chunks.  All weight issues happen in the first ~10us,
                # while those engines have no other work yet — DMA_DIRECT2D
                # costs ~600ns of issue time on the queueing engine, so it
                # must never sit in front of exp/eviction work.
                (eng or nc.scalar).dma_start(
                    w_sb[:, w_off[w] + e * E : w_off[w] + (e + 1) * E],
                    w_dram[w].ap()[e * P : (e + 1) * P, :],
                )

            xpool = ab.enter_context(tc.tile_pool(name="xpool", bufs=2))
            ppool = ab.enter_context(
                tc.tile_pool(name="proj_ps", bufs=5, space="PSUM")
            )

            H = 1024  # half of the j range handled per streamed xT tile

            def xcol(e, c):
                # column of x chunk-e col-c in xh
                return e * H + c

            def load_half(src, h, with_w=None):
                xh = xpool.tile([P, EC * H], F16, tag="xT")
                if with_w is not None:
                    # Startup ramp: deliver in first-touch order at 512-col
                    # granularity so the first 2MB (x cols 0:512 on sync + M
                    # cols 0:512 on scalar) unlocks the first 4 PSUM groups
                    # (~7us of PE work) while the second waves stream behind.
                    # x-h0 delivery is DMA-issue-rate-bound (~600ns/issue),
                    # so the last-consumed chunks e6/e7 ride the gpsimd SWDGE
                    # queue, whose slow (~12us) bring-up matches when the
                    # in-order e-loop reaches them.
                    # 512-col wave granularity on M measured best: 256-col
                    # sub-waves (tried, reverted) double the scalar queue's
                    # ramp-window issue count, and at 64KB per ~0.6us issue
                    # the queue becomes issue-rate-capped (~107 GB/s) — a
                    # net +5us loss despite the earlier (d0-1,ib0) start.
                    for e in range(EC):
                        nc.scalar.dma_start(
                            w_sb[:, w_off[with_w] + e * E
                                 : w_off[with_w] + e * E + 512],
                            w_dram[with_w].ap()[e * P : (e + 1) * P, 0:512],
                        )
                        # e6/e7 ride gpsimd at the same 512-col wave
                        # granularity as the sync chunks: a full-H chunk
                        # makes the wave-1 groups wait for cols 512:1024
                        # they don't need yet (~1.3us PE gap each).
                        (nc.sync if e < EC - 2 else nc.gpsimd).dma_start(
                            xh[:, e * H : e * H + 512],
                            src.ap()[e * P : (e + 1) * P,
                                     h * H : h * H + 512],
                        )
                    for e in range(EC):
                        nc.scalar.dma_start(
                            w_sb[:, w_off[with_w] + e * E + 512
                                 : w_off[with_w] + (e + 1) * E],
                            w_dram[with_w].ap()[e * P : (e + 1) * P, 512:E],
                        )
                        (nc.sync if e < EC - 2 else nc.gpsimd).dma_start(
                            xh[:, e * H + 512 : (e + 1) * H],
                            src.ap()[e * P : (e + 1) * P,
                                     h * H + 512 : (h + 1) * H],
                        )
                else:
                    for e in range(EC):
                        nc.sync.dma_start(
                            xh[:, e * H : (e + 1) * H],
                            src.ap()[e * P : (e + 1) * P, h * H : (h + 1) * H],
                        )
                return xh

            def load_qT(ec_range, eng):
                # x_q needs no projection: DMA it straight into the
                # attention-phase operand slot.  Queued AFTER the
                # projection-critical bytes (HBM bandwidth is shared across
                # queues; order is the only priority control).
                for e in range(*ec_range):
                    eng.dma_start(
                        qT_sb[:, e * I : (e + 1) * I],
                        q_inT.ap()[e * P : (e + 1) * P, :],
                    )

            def proj_T(xh, h, wname, dst, order=None):
                # dst[d, n] = sum_e W[e,d] x[n,e], n in this half
                if order is None:
                    order = [(d, ib) for d in range(EC)
                             for ib in range(H // 512)]
                for d, ib in order:
                    if True:
                        ps = ppool.tile([P, 512], F32, tag="proj")
                        for e in range(EC):
                            nc.tensor.matmul(
                                ps[:],
                                w_sb[:, wcol(wname, e, d * P)
                                     : wcol(wname, e, d * P) + P],
                                xh[:, xcol(e, ib * 512)
                                   : xcol(e, ib * 512) + 512],
                                start=(e == 0),
                                stop=(e == EC - 1),
                            )
                        nc.scalar.activation(
                            dst[:, d * I + h * H + ib * 512
                                : d * I + h * H + (ib + 1) * 512],
                            ps[:],
                            mybir.ActivationFunctionType.Identity,
                        )

            def proj_v(xh, h):
                # v[j, e] = sum_e' k_v[j, e'] Wv[e', e] + bv[e], j in this half
                for jc in range(H // P):
                    jg = h * (H // P) + jc
                    for eh in range(E // 512):
                        ps = ppool.tile([P, 512], F32, tag="proj")
                        for e in range(EC):
                            nc.tensor.matmul(
                                ps[:],
                                xh[:, xcol(e, jc * P)
                                   : xcol(e, jc * P) + P],
                                w_sb[:, wcol("Wv", e, eh * 512)
                                     : wcol("Wv", e, eh * 512) + 512],
                                start=(e == 0),
                                stop=(e == EC - 1),
                            )
                        nc.vector.tensor_add(
                            v_sb[:, jg * EV + eh * 512 : jg * EV + eh * 512 + 512],
                            ps[:],
                            bv_sb[:, eh * 512 : (eh + 1) * 512],
                        )

            # Warmup spin: throwaway matmuls on a zeroed tile keep the PE
            # busy from ~7.5us (engine start) so the HAM activity window
            # fills and the clock gate opens right as the first real
            # operands land (the runtime preamble plus first-chunk transfer
            # make ~13us the floor).  Without this the first ~25us of
            # projections run at half clock.  NOTE: the device's steady
            # clock itself varies per run (measured 1.98 vs 2.37 GHz with
            # identical code — ~70us swing); that state is not controllable
            # from the kernel.
            warm_ps_pool = ab.enter_context(
                tc.tile_pool(name="warm_ps", bufs=1, space="PSUM")
            )
            warm_sb = const.tile([P, 512], F16, tag="warm")
            nc.vector.memset(warm_sb[:], 0.0)
            warm_ps = warm_ps_pool.tile([P, 512], F32, tag="warm")

            def spin(n):
                for _ in range(n):
                    nc.tensor.matmul(
                        warm_ps[:], warm_sb[:, 0:P], warm_sb[:],
                        start=True, stop=True, skip_group_check=True,
                    )

            # DMA priority (HBM bandwidth is shared across queues, FIFO within
            # one): the critical first set — x-h0 on sync+gpsimd + M on scalar
            # — owns the queues from t=0.  Everything else follows in deadline
            # order.  bv (512KB, needed ~65us) queues BEHIND the ramp-critical
            # e6/7 chunks on gpsimd.  proj_T(h1) runs BEFORE proj_v(h0) so the
            # Wv deadline moves from ~35us to ~62us, letting Wv queue behind
            # x-h1.
            xh0 = load_half(k_vT, 0, with_w="M")
            # beta (8KB, needed at first exp ~100us) and bv (512KB, needed
            # ~65us) queue BEHIND the ramp-critical e6/7 chunks on gpsimd —
            # even beta's ~780ns SWDGE issue in front of e6 costs PE time.
            nc.gpsimd.dma_start(beta_sb[:], beta_d.ap())
            nc.gpsimd.dma_start(bv_sb[:], bv_bc.ap())
            xh1 = load_half(k_vT, 1)
            for e in range(EC):
                load_w_chunk("Wv", e, eng=nc.sync)
            load_qT((0, EC), nc.sync)
            # h0 is emitted e-INNER across four simultaneously-open PSUM
            # groups, in data-arrival order (wave 1 = x cols 0:512 + M cols
            # 0:512 serves (d0-3, ib0); wave 2 adds x 512:1024 -> ib1 and M
            # 512:1024 -> d4-7).  Each arriving x/M e-chunk pair immediately
            # yields 4 real matmuls (2048 cycles), so the wave-1 work
            # retires DURING the DMA-bound ramp window instead of after it.
            # Interleaved spins keep the HAM activity duty high between
            # chunk arrivals so the DVFS gate, opened by the lead-in spins
            # at ~13-14us, never re-closes (a closed gate replays matmuls
            # at half clock even with data present).
            def proj_T_set(xh, h, wname, dst, ds, ib, spins_per_e):
                ps = [
                    ppool.tile([P, 512], F32, tag="proj",
                               name=f"ps_{h}_{ib}_{d}")
                    for d in ds
                ]
                for e in range(EC):
                    for i, d in enumerate(ds):
                        nc.tensor.matmul(
                            ps[i][:],
                            w_sb[:, wcol(wname, e, d * P)
                                 : wcol(wname, e, d * P) + P],
                            xh[:, xcol(e, ib * 512)
                               : xcol(e, ib * 512) + 512],
                            start=(e == 0),
                            stop=(e == EC - 1),
                        )
                    if spins_per_e:
                        spin(spins_per_e)
                for i, d in enumerate(ds):
                    nc.scalar.activation(
                        dst[:, d * I + h * H + ib * 512
                            : d * I + h * H + (ib + 1) * 512],
                        ps[i][:],
                        mybir.ActivationFunctionType.Identity,
                    )

            spin(13)
            proj_T_set(xh0, 0, "M", kT_sb, (0, 1, 2, 3), 0, 0)
            proj_T_set(xh0, 0, "M", kT_sb, (0, 1, 2, 3), 1, 0)
            proj_T_set(xh0, 0, "M", kT_sb, (4, 5, 6, 7), 0, 0)
            proj_T_set(xh0, 0, "M", kT_sb, (4, 5, 6, 7), 1, 0)
            proj_T(xh1, 1, "M", kT_sb)
            proj_v(xh0, 0)
            # Pre-emit the first attention block's first two sim+exp groups
            # into the projection tail (they need only kT h0 + qT, both
            # resident) — primes the exp pipeline so attention opens with
            # its PSUM banks already drained and no transition bubble.
            primed = [(jc, emit_sim_exp(0, IB, jc)) for jc in range(2)]
            proj_v(xh1, 1)

        # ---------------- phase C: attention ----------------
        with ExitStack() as c:
            # NOTE: matmul start=True clears has_written for the WHOLE PSUM
            # bank, so each accumulation group needs its own bank.  PV is
            # split 384/384/257 (not 512/512) so the denominator's ones
            # column fits the 2KB bank (513*4 would not); 2 isub * 3 pieces
            # = 6 banks + 2 sim = all 8.
            pv_ps_pool = c.enter_context(
                tc.tile_pool(name="pv_ps", bufs=6, space="PSUM")
            )
            out_pool = c.enter_context(tc.tile_pool(name="outsb", bufs=6))
            small = c.enter_context(tc.tile_pool(name="small", bufs=2))

            # (col0, width) of each PV piece within the padded EV-col v
            # chunk; the last piece's final column is the ones column, so
            # its last PSUM col accumulates the softmax denominator.
            # (A 4-piece final block was tried to narrow the last drain:
            # the extra evict+DMA pair contends on the two drain queues and
            # issues the last DMA ~1us LATER — 3 pieces measured best.)
            PW3 = [(0, 384), (384, 384), (768, 258)]
            NJC = J // P
            # The final 256-row block splits into two 128-row blocks: the
            # penultimate one's drain overlaps the last one's compute, and
            # the unavoidable serial tail chain halves.
            blocks = [(k * IB, IB) for k in range(I // IB - 1)]
            blocks += [(I - IB, P), (I - P, P)]

            for bi, (i0, ibsz) in enumerate(blocks):
                nsub = ibsz // P
                last = bi == len(blocks) - 1
                PW = PW3
                dp = len(PW) - 1  # den-carrying piece index
                dcol = PW[dp][1] - 1  # den column within that piece
                pv = [
                    [
                        pv_ps_pool.tile(
                            [P, w], F32, tag="pv", name=f"pv_{i0}_{s}_{pi}"
                        )
                        for pi, (_, w) in enumerate(PW)
                    ]
                    for s in range(nsub)
                ]

                def emit_pv(jc, expT):
                    for isub in range(nsub):
                        lhs = expT[:, isub * P : (isub + 1) * P]
                        # On the final j-chunk, finish the den-carrying
                        # piece FIRST so the recip -> eviction -> DMA drain
                        # chain overlaps the other PV streams instead of
                        # trailing them.
                        order = range(len(PW))
                        if jc == NJC - 1:
                            order = (dp, *range(dp))
                        for pi in order:
                            c0, w = PW[pi]
                            nc.tensor.matmul(
                                pv[isub][pi][:],
                                lhs,
                                v_sb[:, jc * EV + c0 : jc * EV + c0 + w],
                                start=(jc == 0),
                                stop=(jc == NJC - 1),
                            )

                # pv/den for chunk jc are emitted after sim for chunk
                # jc+2, so the exp -> semaphore -> PE latency hides under
                # two full sim streams instead of poking a ~125ns bubble
                # into each cycle.  Block 0's first two groups were
                # pre-emitted into the projection tail.
                pending = list(primed) if bi == 0 else []
                for jc in range(len(pending), NJC):
                    expT = emit_sim_exp(i0, ibsz, jc)
                    pending.append((jc, expT))
                    if len(pending) > 2:
                        emit_pv(*pending.pop(0))
                for item in pending:
                    emit_pv(*item)

                recip = small.tile([P, nsub], F32, tag="recip")
                for isub in range(nsub):
                    nc.vector.reciprocal(
                        recip[:, isub : isub + 1],
                        pv[isub][dp][:, dcol : dcol + 1],
                    )
                # All evictions on DVE (idle during attention) so the ACT
                # queue stays exp-only — an eviction burst on ACT at a block
                # boundary delays exp(0) and stalls the PE on the sim-bank
                # reuse semaphore.  Exception: the final block has no exp
                # work left, so its evictions alternate DVE/ACT and each
                # piece kicks its own DMA (sync/scalar) as soon as it lands,
                # in piece-completion order (dp first, narrow piece last).
                for isub in range(nsub):
                    if not last:
                        o = out_pool.tile([P, E], F16, tag="o")
                        for pi, (c0, w) in enumerate(PW):
                            wo = w - NONES if pi == dp else w
                            nc.vector.tensor_scalar_mul(
                                o[:, c0 : c0 + wo],
                                pv[isub][pi][:, 0:wo],
                                recip[:, isub : isub + 1],
                            )
                        nc.sync.dma_start(
                            out_d.ap()[i0 + isub * P : i0 + (isub + 1) * P, :],
                            o[:],
                        )
                    else:
                        # Drain lanes alternate DVE+sync / ACT+scalar.  (A
                        # third gpsimd DMA lane was tried for the last
                        # piece: its SWDGE completion latency on the final
                        # barrier cost more than the saved ~0.6us of sync-
                        # queue issue serialization — ~1us net worse.)
                        for k, pi in enumerate((dp, *range(dp))):
                            c0, w = PW[pi]
                            wo = w - NONES if pi == dp else w
                            o = out_pool.tile([P, 384], F16, tag="o2")
                            if k % 2 == 0:
                                nc.vector.tensor_scalar_mul(
                                    o[:, 0:wo], pv[isub][pi][:, 0:wo],
                                    recip[:, isub : isub + 1],
                                )
                            else:
                                nc.scalar.activation(
                                    o[:, 0:wo],
                                    pv[isub][pi][:, 0:wo],
                                    mybir.ActivationFunctionType.Copy,
                                    scale=recip[:, isub : isub + 1],
                                )
                            (nc.sync if k % 2 == 0 else nc.scalar).dma_start(
                                out_d.ap()[
                                    i0 + isub * P : i0 + (isub + 1) * P,
                                    c0 : c0 + wo,
                                ],
                                o[:, 0:wo],
                            )

    nc.compile()
    return nc


def _get_nc():
    if "nc" not in _NC_CACHE:
        _NC_CACHE["nc"] = _build()
    return _NC_CACHE["nc"]


def kernel(q_in, k_v, Wq, bq, Wk, bk, Wv, bv):
    q_in = np.asarray(q_in, dtype=np.float32)
    k_v = np.asarray(k_v, dtype=np.float32)
    Wq = np.asarray(Wq, dtype=np.float32)
    Wk = np.asarray(Wk, dtype=np.float32)
    Wv = np.asarray(Wv, dtype=np.float32)
    bq = np.asarray(bq, dtype=np.float32)
    bv = np.asarray(bv, dtype=np.float32)

    nc = _get_nc()

    # sim = x_q (Wq Wk^T) x_k^T (+ bias terms, see module docstring).
    # proj_T consumes weights in [in, out] layout: W'[e, d] with
    # k'[d] = sum_e W'[e, d] x_k[e], and W' = (Wq Wk^T)^T = Wk Wq^T.
    M_t16 = np.ascontiguousarray((Wk @ Wq.T).astype(np.float16))
    Wv16 = np.ascontiguousarray(Wv.astype(np.float16))
    bv_bc = np.ascontiguousarray(np.broadcast_to(bv, (P, E)))
    u = Wk @ bq  # beta_j = SCALE * x_k[j] . u  (zero when bq == 0)

    in_maps = []
    for b in range(B):
        beta = (SCALE * (k_v[b] @ u)).astype(np.float32)
        beta_t = np.ascontiguousarray(beta.reshape(J // P, P).T)
        in_maps.append(
            {
                "q_inT": np.ascontiguousarray(q_in[b].T).astype(np.float16),
                "k_vT": np.ascontiguousarray(k_v[b].T).astype(np.float16),
                "M_t": M_t16,
                "Wv": Wv16,
                "beta_t": beta_t,
                "bv_bc": bv_bc,
            }
        )

    global LAST_RESULTS
    LAST_RESULTS = run_bass_kernel_spmd(
        nc, in_maps, core_ids=list(range(B)), **_RUN_KWARGS
    )
    return np.stack(
        [LAST_RESULTS.results[b]["out"].astype(np.float32) for b in range(B)]
    )

